# revision 67
# baseline (speedup 1.0000x reference)
"""Causal multi-head self-attention with RoPE on 8 Trainium2 NeuronCores.

Sharding: batch (4) x head-group (2) -> 8 cores (tensor parallel over
heads).  Each core projects K/V/Q for its 8 heads over the full sequence,
runs causal attention for all 2048 queries, and row-shards the output
projection; the two partial [2048, 1024] f32 outputs per batch are summed
on the host (the all-reduce of the TP out-projection).  Unlike batch x
query-split sharding this duplicates no K/V compute and pays no causal
load-balance padding: query tile T needs exactly 2T+2 key blocks.

Per-core structure (f16 data paths, f32 PSUM):
  phase 1, per 512-column seq chunk: K^T/Q^T projections in transposed
    [feat, seq] layout (stationary = weight chunks) and V in natural
    [seq, feat] layout (stationary = X chunks), RoPE as
    cos*x + sin*(P@x) with a constant pair-permutation matmul, software-
    pipelined one tile behind the projections.
  phase 2, per 512-query tile T (issued so tile T overlaps chunk T+1's
    projections): per (head-pair, head, key-block-pair) scores land as
    two [128, 512] one-matmul-per-PSUM-bank groups, one [128, 1024] exp
    on ACT reads both banks, AV^T accumulates [65, 512] with a trailing
    ones column producing the softmax denominator on partition 64.  The
    diagonal key-block-pair computes only the upper query half (exact
    causality); the two preceding blocks use multiplicative f16 masks.
    Normalization (reciprocal -> PE ones-broadcast -> multiply) is
    deferred into the next (head-pair, head) iteration to hide latency;
    the odd head's rows reach aT via an SBUF->SBUF DMA (cross-partition).
  phase 3: out-projection chunks ([128 q, 512] PSUM groups over 4
    head-pair stationaries) are interleaved into tiles 2/3 as PE filler
    for the ACT-bound exp stretches, paced proportionally to key-block
    progress; only tile 3's chunks trail the last normalize.

Engine budget per core (cost model): PE ~249us busy (projections 82,
scores 61, AV 61, out-proj 27, RoPE perms 7, norm broadcast 7), ACT
~185us (exp in [128, 1024] bites), DVE ~145us, in a ~292us kernel.
PSUM (8 banks): proj/perm+out-proj pool 2, scores pool 2x2 (also the
normalize broadcast), AV accumulators 2.  One matmul accumulation group
per bank (start=True resets the whole bank); tile-position packed
multi-region groups are illegal; GPSIMD cannot touch PSUM; DVE ops need
equal start partitions and at most one PSUM operand.
"""

import os
import sys
import math

if "/opt/trn_rl_repo" not in sys.path:
    sys.path.append("/opt/trn_rl_repo")

import numpy as np

import concourse.bass as bass
import concourse.tile as tile
from concourse import bacc, mybir
from concourse.bass_utils import run_bass_kernel_spmd

B = 4
S = 2048
D = 1024
H = 16          # total heads
HC = 8          # heads per core
NEP = HC // 2   # head-pairs per core (128-partition groups)
DK = 64
QT = 512        # query tile
NT = S // QT    # 4 query tiles
ST = 512        # seq chunk for projections
THETA = 10000.0

F32R = mybir.dt.float32r
F32 = mybir.dt.float32
F16 = mybir.dt.float16

_cache = {}


def _build_program():
    if "nc" in _cache:
        return _cache["nc"]

    nc = bacc.Bacc("TRN2")

    xt_d = nc.dram_tensor("xt", [D, S], F16, kind="ExternalInput")
    wkt_d = nc.dram_tensor("wkt", [D, 512], F16, kind="ExternalInput")
    wqt_d = nc.dram_tensor("wqt", [D, 512], F16, kind="ExternalInput")
    wvt_d = nc.dram_tensor("wvt", [D, 512], F16, kind="ExternalInput")
    wot_d = nc.dram_tensor("wot", [512, D], F16, kind="ExternalInput")
    cos_d = nc.dram_tensor("cos", [128, S], F16, kind="ExternalInput")
    sin_d = nc.dram_tensor("sin", [128, S], F16, kind="ExternalInput")
    maska_d = nc.dram_tensor("maska", [128, 2, QT], F16, kind="ExternalInput")
    ones_d = nc.dram_tensor("ones1", [128, DK], F32R, kind="ExternalInput")
    y_d = nc.dram_tensor("y", [S, D], F32, kind="ExternalOutput")

    xt_t = xt_d.rearrange("(n p) s -> p n s", p=128)
    wkt_t = wkt_d.rearrange("(n p) e -> p n e", p=128)
    wqt_t = wqt_d.rearrange("(n p) e -> p n e", p=128)
    wvt_t = wvt_d.rearrange("(n p) e -> p n e", p=128)
    wot_t = wot_d.rearrange("(n p) e -> p n e", p=128)

    with tile.TileContext(nc) as tc:
        with (
            tc.tile_pool(name="const", bufs=1) as cpool,
            tc.tile_pool(name="wpool", bufs=1) as wpool,
            tc.tile_pool(name="kv", bufs=1) as kv,
            tc.tile_pool(name="xs", bufs=2) as xsp,
            tc.tile_pool(name="work", bufs=4) as wk_p,
            tc.tile_pool(name="rope", bufs=6) as rope_p,
            tc.tile_pool(name="ex", bufs=4) as exp_p,
            tc.tile_pool(name="nrm", bufs=6) as nrm_p,
            tc.tile_pool(name="at", bufs=1) as at_p,
            tc.tile_pool(name="outs", bufs=4) as outs_p,
            tc.tile_pool(name="ps1", bufs=2, space="PSUM") as ps1,
            tc.tile_pool(name="psc", bufs=2, space="PSUM") as pscp,
            tc.tile_pool(name="pacc", bufs=1, space="PSUM") as paccp,
        ):
            # ---------------- constants / weights ----------------
            # DMA order matters for the cold start: the first K-projection
            # needs xs(st0) + wk[0] first; everything else follows.
            wk = [wpool.tile([128, 8, 128], F16, tag=f"wk{e}", name=f"wk{e}")
                  for e in range(NEP)]
            wq = [wpool.tile([128, 8, 128], F16, tag=f"wq{e}", name=f"wq{e}")
                  for e in range(NEP)]
            xs0 = xsp.tile([128, 8, ST], F16, tag="xs", name="xs0")
            nc.sync.dma_start(wk[0][:], wkt_t[:, :, 0:128])
            nc.sync.dma_start(xs0[:, 0:2, :], xt_t[:, 0:2, 0:ST])
            nc.gpsimd.dma_start(xs0[:, 4:8, :], xt_t[:, 4:8, 0:ST])
            nc.sync.dma_start(xs0[:, 2:4, :], xt_t[:, 2:4, 0:ST])
            for e in range(1, NEP):
                nc.sync.dma_start(wk[e][:], wkt_t[:, :, e * 128:(e + 1) * 128])
            # cos/sin + Q weights on the gpsimd queue, in parallel
            cosk = cpool.tile([128, S], F16)
            sink = cpool.tile([128, S], F16)
            nc.gpsimd.dma_start(cosk[:], cos_d[:])
            nc.gpsimd.dma_start(sink[:], sin_d[:])
            for e in range(NEP):
                nc.gpsimd.dma_start(wq[e][:], wqt_t[:, :, e * 128:(e + 1) * 128])
            wv = wpool.tile([128, 8, 512], F16)
            nc.sync.dma_start(wv[:], wvt_t[:, :, :])
            ones1 = cpool.tile([128, DK], F32R)
            nc.sync.dma_start(ones1[:], ones_d[:])
            maska = cpool.tile([128, 2, QT], F16)
            nc.sync.dma_start(maska[:], maska_d[:])
            wot = [wpool.tile([128, D], F16, tag=f"wo{e}", name=f"wo{e}")
                   for e in range(NEP)]
            for e in range(NEP):
                nc.sync.dma_start(wot[e][:], wot_t[:, e, :])

            # ---------------- persistent activations ----------------
            krot = [kv.tile([128, S], F16, tag=f"krot{e}", name=f"krot{e}")
                    for e in range(NEP)]
            qrot = [kv.tile([128, S], F16, tag=f"qrot{e}", name=f"qrot{e}")
                    for e in range(NEP)]
            vt = [kv.tile([128, HC, DK + 1], F16, tag=f"vt{kb}",
                          name=f"vt{kb}") for kb in range(S // 128)]
            aT = [at_p.tile([128, QT], F16, tag=f"aT{t}_{e}",
                            name=f"aT{t}_{e}")
                  for t in range(NT) for e in range(NEP)]

            def proj_chunk(st):
                """K/Q/V projection + RoPE for seq columns [512 st, 512 st+512)."""
                if st == 0:
                    xs = xs0
                else:
                    xs = xsp.tile([128, 8, ST], F16, tag="xs")
                    nc.gpsimd.dma_start(xs[:], xt_t[:, :, st * ST:(st + 1) * ST])
                csl = slice(st * ST, (st + 1) * ST)

                def rope_tail(kbf, pp, rot):
                    t_c = rope_p.tile([128, ST], F16, tag="t_c")
                    nc.vector.tensor_mul(t_c[:], kbf[:], cosk[:, csl])
                    t_s = rope_p.tile([128, ST], F16, tag="t_s")
                    nc.vector.tensor_mul(t_s[:], pp[:], sink[:, csl])
                    nc.gpsimd.tensor_add(rot[:, csl], t_c[:], t_s[:])

                # all-K then all-Q, rope muls software-pipelined one behind;
                # the pair permutation is pure partition movement: two
                # stride-2 SBUF->SBUF DMAs (sign lives in the sin table)
                pend_r = None
                for wi, (w, rot) in enumerate(
                        [(wk[e], krot[e]) for e in range(NEP)]
                        + [(wq[e], qrot[e]) for e in range(NEP)]):
                    pk = ps1.tile([128, ST], F32, tag="ps1")
                    for d in range(8):
                        nc.tensor.matmul(pk[:], w[:, d, :], xs[:, d, :],
                                         start=(d == 0), stop=(d == 7))
                    kbf = wk_p.tile([128, ST], F16, tag="kbf")
                    nc.vector.tensor_copy(kbf[:], pk[:])
                    pp = rope_p.tile([128, ST], F16, tag="pp")
                    nc.sync.dma_start(pp[0::2, :], kbf[1::2, :])
                    nc.sync.dma_start(pp[1::2, :], kbf[0::2, :])
                    if pend_r is not None:
                        rope_tail(*pend_r)
                    pend_r = (kbf, pp, rot)
                rope_tail(*pend_r)

                # V projection, natural [seq, feat] layout
                for kb in range(st * (ST // 128), (st + 1) * (ST // 128)):
                    nc.vector.memset(vt[kb][:, :, DK], 1.0)
                for half in range(ST // 128):
                    pv = ps1.tile([128, 512], F32, tag="ps1", name="pv")
                    off = half * 128
                    for d in range(8):
                        nc.tensor.matmul(pv[:], xs[:, d, off:off + 128],
                                         wv[:, d, :], start=(d == 0),
                                         stop=(d == 7))
                    kb = st * (ST // 128) + half
                    dst = vt[kb][:, :, 0:DK]
                    nc.scalar.copy(
                        dst, pv[:].rearrange("p (h w) -> p h w", w=DK))

            pend_norm = [None]

            def normalize(acc, t, e, h):
                """Softmax denominator divide + aT assembly for one (e, h)."""
                lrow = nrm_p.tile([DK + 1, QT], F32R, tag="lrow")
                with nc.allow_low_precision(
                    reason="f32r tile holds full f32 bits"
                ):
                    nc.vector.reciprocal(lrow[DK:DK + 1, :],
                                         acc[DK:DK + 1, :])
                accsb = nrm_p.tile([DK, QT], F16, tag="accsb")
                nc.vector.tensor_copy(accsb[:], acc[0:DK, :])
                pbt = pscp.tile([DK, QT], F32, tag="psc", name="pbt")
                nc.tensor.matmul(pbt[:], ones1[DK:DK + 1, :],
                                 lrow[DK:DK + 1, :], start=True, stop=True)
                rb = nrm_p.tile([DK, QT], F16, tag="rb")
                nc.scalar.copy(rb[:], pbt[:])
                if h == 0:
                    nc.vector.tensor_mul(aT[t * NEP + e][0:DK, :],
                                         accsb[:], rb[:])
                else:
                    tmp = nrm_p.tile([DK, QT], F16, tag="tmp")
                    nc.vector.tensor_mul(tmp[:], accsb[:], rb[:])
                    nc.sync.dma_start(aT[t * NEP + e][DK:128, :], tmp[:])

            E_ORDER = list(range(NEP))

            def out_half(t, qc, et, act_copy=False, sync_dma=False):
                """Output projection for 128 query rows x 512 cols (ps1 pool)."""
                po = ps1.tile([128, 512], F32, tag="ps1", name="po")
                for i, e in enumerate(E_ORDER):
                    nc.tensor.matmul(
                        po[:],
                        aT[t * NEP + e][:, qc * 128:(qc + 1) * 128],
                        wot[e][:, et * 512:(et + 1) * 512],
                        start=(i == 0), stop=(i == NEP - 1),
                    )
                ot = outs_p.tile([128, 512], F32, tag="ot")
                if act_copy:
                    nc.scalar.copy(ot[:], po[:])
                else:
                    nc.vector.tensor_copy(ot[:], po[:])
                r0 = t * QT + qc * 128
                dma = (nc.sync.dma_start if (et == 0 or sync_dma)
                       else nc.gpsimd.dma_start)
                dma(y_d[r0:r0 + 128, et * 512:(et + 1) * 512], ot[:])

            def attn_tile(t, fill=()):
                """Attention for queries [512 t, 512 t + 512).

                fill: prior tiles' out_chunk args emitted at (e, h)
                boundaries as PE filler during ACT-bound stretches.
                """
                fill = list(fill)
                nfill = len(fill)
                qsl = slice(t * QT, (t + 1) * QT)
                qsl2 = slice(t * QT + 256, (t + 1) * QT)  # diagonal half
                nkbp = 2 * t + 2   # key-block pairs (256 keys each)
                total_kbp = nkbp * 2 * NEP
                done = 0
                for e in E_ORDER:
                    for h in (1, 0):
                        hh = 2 * e + h
                        pb = h * DK
                        acc = paccp.tile([DK + 1, QT], F32, tag=f"acc{h}",
                                         name=f"acc{h}")

                        def do_av(ex, c):
                            half = c == nkbp - 1
                            dst = acc[:, 256:QT] if half else acc[:]
                            for j in range(2):
                                nc.tensor.matmul(
                                    dst, vt[2 * c + j][:, hh, :], ex[:, j, :],
                                    start=(c == 0 and j == 0),
                                    stop=(c == nkbp - 1 and j == 1),
                                )

                        pend_av = None
                        for c in range(nkbp):
                            # last pair: only the upper query half is unmasked
                            half = c == nkbp - 1
                            qw = 256 if half else QT
                            qs = qsl2 if half else qsl
                            psc = pscp.tile([128, 2, qw], F32, tag="psc")
                            for j in range(2):
                                kb = 2 * c + j
                                nc.tensor.matmul(
                                    psc[:, j, :],
                                    krot[e][pb:pb + DK, kb * 128:(kb + 1) * 128],
                                    qrot[e][pb:pb + DK, qs],
                                    start=True, stop=True,
                                    tile_position=(pb, 0),
                                )
                            ex = exp_p.tile([128, 2, qw], F16, tag="ex")
                            nc.scalar.activation(
                                ex[:], psc[:],
                                mybir.ActivationFunctionType.Exp,
                                scale=1.0 / math.sqrt(DK),
                            )
                            if c >= nkbp - 2:
                                msk = maska[:, :, 0:qw]
                                exm = exp_p.tile([128, 2, qw], F16, tag="exm")
                                nc.vector.tensor_mul(exm[:], ex[:], msk)
                                ex = exm
                            if pend_av is not None:
                                do_av(*pend_av)
                            pend_av = (ex, c)
                            if c == min(1, nkbp - 1) and pend_norm[0]:
                                normalize(*pend_norm[0])
                                pend_norm[0] = None
                            done += 1
                        budget = done * nfill // total_kbp
                        if fill and nfill - len(fill) < budget:
                            out_half(*fill.pop(0))
                        do_av(*pend_av)
                        while fill and nfill - len(fill) < budget:
                            out_half(*fill.pop(0))
                        pend_norm[0] = (acc, t, e, h)
                for f in fill:
                    out_half(*f)

            # -------- pipelined schedule --------
            proj_chunk(0)
            proj_chunk(1)
            attn_tile(0)
            proj_chunk(2)
            attn_tile(1)
            proj_chunk(3)
            attn_tile(2, fill=[(0, qc, et) for qc in range(4)
                               for et in range(2)])
            attn_tile(3, fill=[(1, qc, et) for qc in range(4)
                               for et in range(2)]
                      + [(2, qc, et) for qc in range(4) for et in range(2)])
            normalize(*pend_norm[0])
            pend_norm[0] = None
            for qc in range(QT // 128):
                for et in range(2):
                    out_half(NT - 1, qc, et, act_copy=(et == 1),
                             sync_dma=True)

    nc.compile()
    nc.finalize()
    _cache["nc"] = nc
    return nc


def _rope_tables(pos):
    """cos/sin in [128, S] transposed head-pair layout (row r -> pair (r%64)//2)."""
    k = np.arange(DK // 2, dtype=np.float32)
    inv_freq = (THETA ** (-2.0 * k / DK)).astype(np.float32)
    ang = inv_freq[:, None] * pos.astype(np.float32)[None, :]   # [32, S]
    cos64 = np.repeat(np.cos(ang), 2, axis=0)
    sin64 = np.repeat(np.sin(ang), 2, axis=0)
    sin64[0::2, :] *= -1.0   # sign of the pair swap lives here
    return (np.ascontiguousarray(np.concatenate([cos64, cos64], 0)).astype(np.float16),
            np.ascontiguousarray(np.concatenate([sin64, sin64], 0)).astype(np.float16))


def _masks():
    """maska [128, 2, 512] f16: diagonal key-block-pair masks."""
    p = np.arange(128)[:, None]
    f = np.arange(QT)[None, :]
    tris = [(f >= p + 128 * j).astype(np.float16) for j in range(2)]
    return np.ascontiguousarray(np.stack(tris, axis=1))


def _host_inputs(in_features, token_positions, Wq, Wk, Wv, Wo):
    X = np.asarray(in_features, dtype=np.float32)
    pos = np.asarray(token_positions)
    cos, sin = _rope_tables(pos)
    maska = _masks()

    Wqf = np.asarray(Wq, np.float32)
    Wkf = np.asarray(Wk, np.float32)
    Wvf = np.asarray(Wv, np.float32)
    Wof = np.asarray(Wo, np.float32)

    in_maps = []
    for core in range(8):
        b, hg = core // 2, core % 2
        rows = slice(hg * 512, (hg + 1) * 512)
        in_maps.append({
            "xt": np.ascontiguousarray(X[b].T).astype(np.float16),
            "wkt": np.ascontiguousarray(Wkf[rows, :].T).astype(np.float16),
            "wqt": np.ascontiguousarray(Wqf[rows, :].T).astype(np.float16),
            "wvt": np.ascontiguousarray(Wvf[rows, :].T).astype(np.float16),
            "wot": np.ascontiguousarray(Wof[:, rows].T).astype(np.float16),
            "cos": cos, "sin": sin,
            "maska": maska,
            "ones1": np.ones((128, DK), np.float32),
        })
    return in_maps


def kernel(in_features, token_positions, Wq, Wk, Wv, Wo):
    nc = _build_program()
    in_maps = _host_inputs(in_features, token_positions, Wq, Wk, Wv, Wo)

    trace = bool(int(os.environ.get("KERNEL_TRACE", "0")))
    res = run_bass_kernel_spmd(nc, in_maps, core_ids=list(range(8)), trace=trace)
    kernel.last_result = res

    out = np.empty((B, S, D), np.float32)
    for b in range(B):
        out[b] = res.results[2 * b]["y"] + res.results[2 * b + 1]["y"]
    return out
